# revision 2
# baseline (speedup 1.0000x reference)
"""Trainium2 Bass kernel for the segmented block-diagonal linear layer.

out[b, (seg, v, i)] = sum_u x[b, (seg, u, i)] * W_seg[u, v] / sqrt(mu_seg)

Segments (mul_in, mul_out, ir_dim): (256,256,1) (128,128,3) (64,64,5) (32,32,7)
x: [100000, 1184] f32, weight: [1, 87040] f32 -> out: [100000, 1184] f32

Strategy: data-parallel over 8 NeuronCores (12500 rows each). Per core,
stream 128-row tiles; PE-transpose the tile so features land on partitions;
matmul against host-prepared dense block-diagonal weight chunks (the
delta-interleave over the irrep dim is baked into zeros host-side), which
makes every matmul contiguous with free dim >= 256 (full fp32r rate) and
makes outputs land directly in the final feature order.
"""

import sys

if "/opt/trn_rl_repo" not in sys.path:
    sys.path.insert(0, "/opt/trn_rl_repo")

import numpy as np

import concourse.bacc as bacc
import concourse.mybir as mybir
from concourse import masks, tile
from concourse.bass_utils import run_bass_kernel_spmd

SEGS = [(256, 256, 1), (128, 128, 3), (64, 64, 5), (32, 32, 7)]
IN_DIM = 1184
W_NUMEL = 87040
N_CORES = 8
TILE_ROWS = 128

# Transpose pieces: contiguous feature chunks of x (feat_lo, width).
# Piece p is staged at xT columns [128*p, 128*p + width), partitions [0, width).
PIECES = [
    (0, 128), (128, 128),                      # seg0 (256 feats)
    (256, 128), (384, 128), (512, 128),        # seg1 (384 feats)
    (640, 128), (768, 128), (896, 64),         # seg2 (320 feats)
    (960, 128), (1088, 96),                    # seg3 (224 feats)
]

# Segment matmul plan: (piece_indices, psum_bank, psum_col_lo, n_cols, out_feat_lo, out_width)
# seg3 shares bank b0 with seg0 (cols 256:512, padded to N=256 for full fp32r rate).
SEG_PLAN = [
    ([0, 1], "b0", 0, 256, 0, 256),
    ([2, 3, 4], "b1", 0, 384, 256, 384),
    ([5, 6, 7], "b2", 0, 320, 640, 320),
    ([8, 9], "b0", 256, 256, 960, 224),
]

_BUILD_CACHE = {}


def _prepare_dense_weights(weight):
    """Host-side: expand the flat weight into dense per-segment block matrices
    D[u*d+i, v*d+j] = W[u,v] * (i==j) / sqrt(mu), split into <=128-row chunks
    (seg3 column-padded to 256)."""
    w = np.asarray(weight, dtype=np.float32).reshape(-1)
    chunks = []
    off = 0
    for si, (mu, mv, d) in enumerate(SEGS):
        W = w[off : off + mu * mv].reshape(mu, mv) * np.float32(1.0 / np.sqrt(mu))
        off += mu * mv
        D = np.zeros((mu * d, mv * d), dtype=np.float32)
        for i in range(d):
            D[i::d, i::d] = W
        if si == 3:
            Dp = np.zeros((mu * d, 256), dtype=np.float32)
            Dp[:, : mv * d] = D
            D = Dp
        for r0 in range(0, D.shape[0], 128):
            chunks.append(np.ascontiguousarray(D[r0 : r0 + 128]))
    return chunks  # 10 chunks, aligned with PIECES order


def _build(rows_per_core, w_shapes):
    key = (rows_per_core, tuple(w_shapes))
    if key in _BUILD_CACHE:
        return _BUILD_CACHE[key]

    f32 = mybir.dt.float32
    f32r = mybir.dt.float32r

    nc = bacc.Bacc("TRN2", target_bir_lowering=False, debug=False)
    x_d = nc.declare_dram_parameter("x", [rows_per_core, IN_DIM], f32, isOutput=False)
    w_d = [
        nc.declare_dram_parameter(f"wd{i}", list(s), f32, isOutput=False)
        for i, s in enumerate(w_shapes)
    ]
    y_d = nc.declare_dram_parameter("y", [rows_per_core, IN_DIM], f32, isOutput=True)

    n_full, rem = divmod(rows_per_core, TILE_ROWS)
    tiles = [TILE_ROWS] * n_full + ([rem] if rem else [])

    with tile.TileContext(nc) as tc:
        with (
            tc.tile_pool(name="wpool", bufs=1) as wpool,
            tc.tile_pool(name="xpool", bufs=3) as xpool,
            tc.tile_pool(name="xtpool", bufs=2) as xtpool,
            tc.tile_pool(name="ypool", bufs=3) as ypool,
            tc.tile_pool(name="stagp", bufs=2, space="PSUM") as stagp,
            tc.tile_pool(name="outp", bufs=2, space="PSUM") as outp,
        ):
            ident = wpool.tile([128, 128], f32)
            masks.make_identity(nc, ident[:])
            wts = []
            for i, s in enumerate(w_shapes):
                wt = wpool.tile(list(s), f32r, name=f"wsb{i}")
                nc.gpsimd.dma_start(out=wt[:], in_=w_d[i][:, :])
                wts.append(wt)

            r0 = 0
            for rows in tiles:
                xt = xpool.tile([128, IN_DIM], f32, name="xt")
                nc.sync.dma_start(out=xt[:rows, :], in_=x_d[r0 : r0 + rows, :])

                # Transpose pieces into PSUM staging (4 pieces per 512-wide stage),
                # then DVE-copy (with f32->f32r rounding) into the xT sbuf tile.
                xT = xtpool.tile([128, 128 * len(PIECES)], f32r, name="xT")
                for g0 in range(0, len(PIECES), 4):
                    group = PIECES[g0 : g0 + 4]
                    stag = stagp.tile([128, 512], f32, name="stag")
                    for k, (flo, width) in enumerate(group):
                        nc.tensor.transpose(
                            stag[:width, k * 128 : k * 128 + rows],
                            xt[:rows, flo : flo + width],
                            ident[:rows, :rows],
                        )
                    ncols = len(group) * 128
                    nc.vector.tensor_copy(
                        xT[:, g0 * 128 : g0 * 128 + ncols], stag[:, :ncols]
                    )

                # Per-segment dense matmuls, accumulating over feature chunks.
                pb = {
                    "b0": outp.tile([128, 512], f32, name="pb0"),
                    "b1": outp.tile([128, 384], f32, name="pb1"),
                    "b2": outp.tile([128, 320], f32, name="pb2"),
                }
                for pcs, bank, clo, n, _flo, _fw in SEG_PLAN:
                    for j, p in enumerate(pcs):
                        width = PIECES[p][1]
                        nc.tensor.matmul(
                            pb[bank][:rows, clo : clo + n],
                            xT[:width, p * 128 : p * 128 + rows],
                            wts[p][:width, :n],
                            start=(j == 0),
                            stop=(j == len(pcs) - 1),
                        )

                # Copy PSUM -> SBUF output tile (ACT engine), then DMA out.
                yt = ypool.tile([128, IN_DIM], f32, name="yt")
                for _pcs, bank, clo, _n, flo, fw in SEG_PLAN:
                    nc.scalar.copy(
                        out=yt[:rows, flo : flo + fw],
                        in_=pb[bank][:rows, clo : clo + fw],
                    )
                nc.sync.dma_start(out=y_d[r0 : r0 + rows, :], in_=yt[:rows, :])
                r0 += rows

    nc.compile()
    _BUILD_CACHE[key] = nc
    return nc


def _run(x, weight, trace=False, trace_kwargs=None):
    x = np.ascontiguousarray(np.asarray(x, dtype=np.float32))
    batch = x.shape[0]
    assert batch % N_CORES == 0, f"batch {batch} not divisible by {N_CORES}"
    rows_per_core = batch // N_CORES

    wchunks = _prepare_dense_weights(weight)
    nc = _build(rows_per_core, [c.shape for c in wchunks])

    in_maps = []
    for c in range(N_CORES):
        m = {"x": x[c * rows_per_core : (c + 1) * rows_per_core]}
        for i, wc in enumerate(wchunks):
            m[f"wd{i}"] = wc
        in_maps.append(m)

    kwargs = {}
    if trace:
        kwargs["trace"] = True
        if trace_kwargs:
            kwargs["trace_kwargs"] = trace_kwargs
    res = run_bass_kernel_spmd(nc, in_maps, list(range(N_CORES)), **kwargs)
    out = np.concatenate([res.results[c]["y"] for c in range(N_CORES)], axis=0)
    return out.astype(np.float32, copy=False), res


def kernel(x, weight):
    out, _ = _run(x, weight)
    return out


# revision 3
# speedup vs baseline: 1.3380x; 1.3380x over previous
"""Trainium2 Bass kernel for the segmented block-diagonal linear layer.

out[b, (seg, v, i)] = sum_u x[b, (seg, u, i)] * W_seg[u, v] / sqrt(mu_seg)

Segments (mul_in, mul_out, ir_dim): (256,256,1) (128,128,3) (64,64,5) (32,32,7)
x: [100000, 1184] f32, weight: [1, 87040] f32 -> out: [100000, 1184] f32

Strategy: data-parallel over 8 NeuronCores (12500 rows each). Per core,
stream 128-row tiles; PE-transpose the tile so features land on partitions;
matmul against host-prepared dense block-diagonal weight chunks (the
delta-interleave over the irrep dim is baked into zeros host-side), which
makes every matmul contiguous and makes outputs land directly in the final
feature order. Compute dtype is fp16 (cast during the DMA load): the PE's
fp32r mode is TF32-class (~10-bit mantissa) so fp16 matches its accuracy
while streaming 2x faster and transposing in one pass instead of two.
HBM traffic stays fp32 on both sides (the memory roofline is unchanged).
"""

import sys

if "/opt/trn_rl_repo" not in sys.path:
    sys.path.insert(0, "/opt/trn_rl_repo")

import numpy as np

import concourse.bacc as bacc
import concourse.mybir as mybir
from concourse import masks, tile
from concourse.bass_utils import run_bass_kernel_spmd

SEGS = [(256, 256, 1), (128, 128, 3), (64, 64, 5), (32, 32, 7)]
IN_DIM = 1184
W_NUMEL = 87040
N_CORES = 8
TILE_ROWS = 128

# Transpose pieces: contiguous feature chunks of x (feat_lo, width).
# Piece p is staged at xT columns [128*p, 128*p + width), partitions [0, width).
PIECES = [
    (0, 128), (128, 128),                      # seg0 (256 feats)
    (256, 128), (384, 128), (512, 128),        # seg1 (384 feats)
    (640, 128), (768, 128), (896, 64),         # seg2 (320 feats)
    (960, 128), (1088, 96),                    # seg3 (224 feats)
]

# Segment matmul plan: (piece_indices, psum_bank, psum_col_lo, n_cols, out_feat_lo, out_width)
# seg3 shares bank b0 with seg0 (cols 256:512, column-padded to 256).
SEG_PLAN = [
    ([0, 1], "b0", 0, 256, 0, 256),
    ([2, 3, 4], "b1", 0, 384, 256, 384),
    ([5, 6, 7], "b2", 0, 320, 640, 320),
    ([8, 9], "b0", 256, 256, 960, 224),
]

_BUILD_CACHE = {}


def _prepare_dense_weights(weight):
    """Host-side: expand the flat weight into dense per-segment block matrices
    D[u*d+i, v*d+j] = W[u,v] * (i==j) / sqrt(mu), split into <=128-row chunks
    (seg3 column-padded to 256), cast to fp16 for the PE."""
    w = np.asarray(weight, dtype=np.float32).reshape(-1)
    chunks = []
    off = 0
    for si, (mu, mv, d) in enumerate(SEGS):
        W = w[off : off + mu * mv].reshape(mu, mv) * np.float32(1.0 / np.sqrt(mu))
        off += mu * mv
        D = np.zeros((mu * d, mv * d), dtype=np.float32)
        for i in range(d):
            D[i::d, i::d] = W
        if si == 3:
            Dp = np.zeros((mu * d, 256), dtype=np.float32)
            Dp[:, : mv * d] = D
            D = Dp
        for r0 in range(0, D.shape[0], 128):
            chunks.append(np.ascontiguousarray(D[r0 : r0 + 128]).astype(np.float16))
    return chunks  # 10 chunks, aligned with PIECES order


def _build(rows_per_core, w_shapes):
    key = (rows_per_core, tuple(w_shapes))
    if key in _BUILD_CACHE:
        return _BUILD_CACHE[key]

    f32 = mybir.dt.float32
    f16 = mybir.dt.float16

    nc = bacc.Bacc("TRN2", target_bir_lowering=False, debug=False)
    x_d = nc.declare_dram_parameter("x", [rows_per_core, IN_DIM], f32, isOutput=False)
    w_d = [
        nc.declare_dram_parameter(f"wd{i}", list(s), f16, isOutput=False)
        for i, s in enumerate(w_shapes)
    ]
    y_d = nc.declare_dram_parameter("y", [rows_per_core, IN_DIM], f32, isOutput=True)

    n_full, rem = divmod(rows_per_core, TILE_ROWS)
    tiles = [TILE_ROWS] * n_full + ([rem] if rem else [])

    with tile.TileContext(nc) as tc:
        with (
            tc.tile_pool(name="wpool", bufs=1) as wpool,
            tc.tile_pool(name="xpool", bufs=3) as xpool,
            tc.tile_pool(name="xtpool", bufs=2) as xtpool,
            tc.tile_pool(name="ypool", bufs=3) as ypool,
            tc.tile_pool(name="stagp", bufs=2, space="PSUM") as stagp,
            tc.tile_pool(name="outp", bufs=2, space="PSUM") as outp,
        ):
            ident = wpool.tile([128, 128], f16)
            masks.make_identity(nc, ident[:])
            wts = []
            for i, s in enumerate(w_shapes):
                wt = wpool.tile(list(s), f16, name=f"wsb{i}")
                nc.sync.dma_start(out=wt[:], in_=w_d[i][:, :])
                wts.append(wt)

            r0 = 0
            for rows in tiles:
                # fp32 -> fp16 cast during the DMA load (SWDGE); HBM still reads fp32
                xt = xpool.tile([128, IN_DIM], f16, name="xt")
                nc.gpsimd.dma_start(out=xt[:rows, :], in_=x_d[r0 : r0 + rows, :])

                # Transpose pieces into PSUM staging (4 pieces per 512-wide stage),
                # then DVE-copy into the xT sbuf tile.
                xT = xtpool.tile([128, 128 * len(PIECES)], f16, name="xT")
                for g0 in range(0, len(PIECES), 4):
                    group = PIECES[g0 : g0 + 4]
                    stag = stagp.tile([128, 512], f16, name="stag")
                    for k, (flo, width) in enumerate(group):
                        nc.tensor.transpose(
                            stag[:width, k * 128 : k * 128 + rows],
                            xt[:rows, flo : flo + width],
                            ident[:rows, :rows],
                        )
                    ncols = len(group) * 128
                    nc.vector.tensor_copy(
                        xT[:, g0 * 128 : g0 * 128 + ncols], stag[:, :ncols]
                    )

                # Per-segment dense matmuls, accumulating over feature chunks.
                pb = {
                    "b0": outp.tile([128, 512], f32, name="pb0"),
                    "b1": outp.tile([128, 384], f32, name="pb1"),
                    "b2": outp.tile([128, 320], f32, name="pb2"),
                }
                for pcs, bank, clo, n, _flo, _fw in SEG_PLAN:
                    for j, p in enumerate(pcs):
                        width = PIECES[p][1]
                        nc.tensor.matmul(
                            pb[bank][:rows, clo : clo + n],
                            xT[:width, p * 128 : p * 128 + rows],
                            wts[p][:width, :n],
                            start=(j == 0),
                            stop=(j == len(pcs) - 1),
                        )

                # Copy PSUM -> SBUF output tile (ACT engine), then DMA out.
                yt = ypool.tile([128, IN_DIM], f32, name="yt")
                for _pcs, bank, clo, _n, flo, fw in SEG_PLAN:
                    nc.scalar.copy(
                        out=yt[:rows, flo : flo + fw],
                        in_=pb[bank][:rows, clo : clo + fw],
                    )
                nc.sync.dma_start(out=y_d[r0 : r0 + rows, :], in_=yt[:rows, :])
                r0 += rows

    nc.compile()
    _BUILD_CACHE[key] = nc
    return nc


def _run(x, weight, trace=False, trace_kwargs=None):
    x = np.ascontiguousarray(np.asarray(x, dtype=np.float32))
    batch = x.shape[0]
    assert batch % N_CORES == 0, f"batch {batch} not divisible by {N_CORES}"
    rows_per_core = batch // N_CORES

    wchunks = _prepare_dense_weights(weight)
    nc = _build(rows_per_core, [c.shape for c in wchunks])

    in_maps = []
    for c in range(N_CORES):
        m = {"x": x[c * rows_per_core : (c + 1) * rows_per_core]}
        for i, wc in enumerate(wchunks):
            m[f"wd{i}"] = wc
        in_maps.append(m)

    kwargs = {}
    if trace:
        kwargs["trace"] = True
        if trace_kwargs:
            kwargs["trace_kwargs"] = trace_kwargs
    res = run_bass_kernel_spmd(nc, in_maps, list(range(N_CORES)), **kwargs)
    out = np.concatenate([res.results[c]["y"] for c in range(N_CORES)], axis=0)
    return out.astype(np.float32, copy=False), res


def kernel(x, weight):
    out, _ = _run(x, weight)
    return out
